# revision 13
# baseline (speedup 1.0000x reference)
"""Trainium2 Bass kernel for nn_MixSizeNumQuatEmbedding (vq_codebook).

Data-parallel over batch across 8 NeuronCores. Per core (512 batch rows,
N = 512*39 = 19968 lookups, lookup order n = f*512 + b):

  stage A (codeword ids): one dma_gather per field pulls the 256B-padded
    row of the column-concatenated index table (int16) for each of the
    field's 512 global feature ids (x < 10000 fits int16); an SBUF->SBUF
    fold DMA plus per-column DVE copies rewrap the 15 codeword-id streams
    into dma_gather's 16-partition-interleaved index format. (Optional:
    cw_on_device=False moves this stage to host numpy.)

  stage B (embedding gather): per (codebook, subvector) stream s, chunked
    dma_gathers pull one subvector row slice per lookup straight from HBM
    (host uploads per-subvector codebook column slices so every gathered
    element is a legal 256B/512B row).

  stage C: DVE multiplies each gathered slice by its per-field arch_prob
    weight (free-dim broadcast AP) and accumulates the 7 codebook
    contributions into a [128, Tc, 128] tile, stored contiguously per
    partition to DRAM.
"""

import numpy as np

import concourse.bacc as bacc
import concourse.bass as bass
import concourse.mybir as mybir
import concourse.tile as tile

# Problem constants (hardcoded per harness contract).
B, F, D = 4096, 39, 128
FIELD_DIM = 10000
N_CORES = 8
BC = B // N_CORES            # 512 batch rows per core
N = BC * F                   # 19968 lookups per core
T = N // 128                 # 156 slots of 128 lookups
NJ = N // 16                 # 1248 wrapped-index columns
G = F * FIELD_DIM            # 390000 global feature ids

PAIRS = [(0, 0), (1, 0), (1, 1), (1, 2), (2, 0), (2, 1), (2, 2)]
M_SPACE = [1, 2, 4]
ROWS = {0: 19500, 1: 9984, 2: 19968}
PAIR_ROWS = [ROWS[j] for (j, m) in PAIRS]
PAIR_MN = [M_SPACE[m] for (j, m) in PAIRS]
PAIR_APCOL = [j * 3 + m for (j, m) in PAIRS]

# 15 (codebook, subvector) gather streams. Stream s gathers `w` floats per
# lookup from its host-sliced codebook view hcb_s [rows, w]; the first
# `plen` are valid and scale-accumulate into acc[:, :, doff:doff+plen].
SLICES = []
for p, (j, m) in enumerate(PAIRS):
    mn = PAIR_MN[p]
    plen = D // mn
    w = 128 if mn == 1 else 64
    for i in range(mn):
        SLICES.append(dict(pair=p, sub=i, w=w, plen=plen, doff=i * plen))
NS = len(SLICES)
assert NS == 15

_CACHE = {}


def build_program(loop=0, cw_on_device=True, tc=8, nqueues=4, scratch=16384,
                  skip_fma=False, skip_store=False, gbufs=2):
    """loop>0: wrap the body in a HW For_i loop (timing amplification)."""
    key = (loop, cw_on_device, tc, nqueues, scratch, skip_fma, skip_store,
           gbufs)
    if key in _CACHE:
        return _CACHE[key]
    f32 = mybir.dt.float32
    i16 = mybir.dt.int16
    chunks = [(t0, min(tc, T - t0)) for t0 in range(0, T, tc)]
    nc = bacc.Bacc("TRN2", target_bir_lowering=False, debug=False,
                   num_devices=N_CORES, num_swdge_queues=nqueues,
                   dynamic_dma_scratch_size=scratch)

    hcb_d = [nc.dram_tensor(f"hcb{s}", [PAIR_ROWS[sl["pair"]], sl["w"]], f32,
                            kind="ExternalInput")
             for s, sl in enumerate(SLICES)]
    sc_d = nc.dram_tensor("scmap", [128, 7, T], f32, kind="ExternalInput")
    out_d = nc.dram_tensor("out", [128, T * D], f32, kind="ExternalOutput")
    if cw_on_device:
        # x16[p, f, j4] = x[b = 16*j4 + p%16, f] wrapped for per-field gathers
        x16_d = nc.dram_tensor("x16", [128, F, BC // 16], i16,
                               kind="ExternalInput")
        idx16_d = nc.dram_tensor("idx16", [G, 128], i16, kind="ExternalInput")
    else:
        iall_d = nc.dram_tensor("iall", [128, NS, NJ], i16,
                                kind="ExternalInput")

    from contextlib import ExitStack
    with tile.TileContext(nc) as tc_ctx, ExitStack() as ctx:
        cpool = ctx.enter_context(tc_ctx.tile_pool(name="const", bufs=1))
        gpool = ctx.enter_context(tc_ctx.tile_pool(name="g", bufs=gbufs))
        apool = ctx.enter_context(tc_ctx.tile_pool(name="acc", bufs=2))
        tpool = ctx.enter_context(tc_ctx.tile_pool(name="tmp", bufs=2))

        sc_sb = cpool.tile([128, 7, T], f32)
        nc.sync.dma_start(out=sc_sb[:], in_=sc_d.ap())

        iall = cpool.tile([128, NS, NJ], i16)
        if cw_on_device:
            x16 = cpool.tile([128, F, BC // 16], i16)
            nc.sync.dma_start(out=x16[:], in_=x16_d.ap())
        else:
            nc.sync.dma_start(out=iall[:], in_=iall_d.ap())

        out_ap = out_d.ap().rearrange("p (t d) -> p t d", d=D)

        FC = 13                 # fields per stage-A chunk (SBUF economy)

        def body():
            if cw_on_device:
              for fc in range(0, F, FC):
                nf = min(FC, F - fc)
                # cw: per field f, position b -> partition b%128, slot b//128
                cw = cpool.tile([128, FC, 4, 128], i16, tag="cw")
                for fi in range(nf):
                    f = fc + fi
                    nc.gpsimd.dma_gather(
                        out_ap=cw[:, fi, :, :],
                        in_ap=idx16_d.ap()[f * FIELD_DIM:(f + 1) * FIELD_DIM, :],
                        idxs_ap=x16[:, f, :],
                        num_idxs=BC, num_idxs_reg=BC, elem_size=128,
                        queue_num=f % nqueues)
                # fold 16g+r -> r: fd[r, fi, s4, g, c] = cw[16g+r, fi, s4, c]
                fd = cpool.tile([128, FC, 4, 8, 16], i16, tag="fd")
                for g in range(8):
                    nc.sync.dma_start(
                        out=fd[0:16, 0:nf, :, g, :],
                        in_=cw[16 * g:16 * (g + 1), 0:nf, :, 0:16])
                # wrapped index rows: iall[r, s, 32f+8s4+g] = cw_s(16j+r)
                fdv = fd[0:16, 0:nf, :, :, :].rearrange(
                    "r f a g c -> r (f a g) c")
                for s in range(NS):
                    nc.vector.tensor_copy(
                        out=iall[0:16, s, fc * 32:(fc + nf) * 32],
                        in_=fdv[:, :, s])
              # replicate to the other 7 16-partition groups (tree doubling)
              for span in (16, 32, 64):
                nc.sync.dma_start(out=iall[span:2 * span, :, :],
                                  in_=iall[0:span, :, :])

            for t0, tc_ in chunks:
              gts = []
              for s, sl in enumerate(SLICES):
                gt = gpool.tile([128, tc, sl["w"]], f32, tag=f"g{s}")
                nc.gpsimd.dma_gather(
                    out_ap=gt[:, 0:tc_, :],
                    in_ap=hcb_d[s].ap(),
                    idxs_ap=iall[:, s, t0 * 8:(t0 + tc_) * 8],
                    num_idxs=tc_ * 128, num_idxs_reg=tc_ * 128,
                    elem_size=sl["w"], queue_num=s % nqueues)
                gts.append(gt)

              acc = apool.tile([128, tc, D], f32)
              for s, sl in enumerate(SLICES if not skip_fma else []):
                plen = sl["plen"]
                gv = gts[s][:, 0:tc_, 0:plen]
                sc_bc = (sc_sb[:, sl["pair"], t0:t0 + tc_]
                         .unsqueeze(2).to_broadcast([128, tc_, plen]))
                dst = acc[:, 0:tc_, sl["doff"]:sl["doff"] + plen]
                if s == 0:
                    nc.vector.tensor_tensor(out=dst, in0=gv, in1=sc_bc,
                                            op=mybir.AluOpType.mult)
                else:
                    tmp = tpool.tile([128, tc, plen], f32, tag="tmp")
                    nc.vector.tensor_tensor(out=tmp[:, 0:tc_, :], in0=gv,
                                            in1=sc_bc,
                                            op=mybir.AluOpType.mult)
                    nc.vector.tensor_tensor(out=dst, in0=dst,
                                            in1=tmp[:, 0:tc_, :],
                                            op=mybir.AluOpType.add)
              if not skip_store and not skip_fma:
                nc.sync.dma_start(out=out_ap[:, t0:t0 + tc_, :],
                                  in_=acc[:, 0:tc_, :])
              elif not skip_store:
                # keep the output written so the program has live results
                nc.sync.dma_start(
                    out=out_ap[:, t0:t0 + tc_, :],
                    in_=gts[1][:, 0:tc_, :])

        if loop:
            with tc_ctx.For_i(0, loop, 1):
                body()
        else:
            body()

    nc.compile()
    _CACHE[key] = nc
    return nc


def host_prep(inputs, cw_on_device=True):
    """Build per-core in_maps from the full problem inputs."""
    x = np.asarray(inputs["x"])
    arch_prob = np.asarray(inputs["arch_prob"], dtype=np.float32)

    idx_cols = []
    for (j, m) in PAIRS:
        idx_cols.append(np.asarray(inputs[f"idx_{j}_{m}"]).astype(np.int16))
    idxcat = np.concatenate(idx_cols, axis=1)                     # [G, 15]

    shared = {}
    if cw_on_device:
        idx16 = np.zeros((G, 128), np.int16)
        idx16[:, :15] = idxcat
        shared["idx16"] = idx16

    for s, sl in enumerate(SLICES):
        (j, m) = PAIRS[sl["pair"]]
        cb = np.asarray(inputs[f"cb_{j}_{m}"]).astype(np.float32)
        mn = PAIR_MN[sl["pair"]]
        i, plen, w = sl["sub"], sl["plen"], sl["w"]
        if mn == 1:
            hv = cb
        else:
            hv = np.zeros((cb.shape[0], w), np.float32)
            take = min(w, D - i * plen)
            hv[:, :take] = cb[:, i * plen:i * plen + take]
        shared[f"hcb{s}"] = np.ascontiguousarray(hv)

    # scale map: scmap[p, pair, t] = arch_prob[t//4, apcol(pair)]
    s_pair_f = arch_prob[:, PAIR_APCOL].T.astype(np.float32)      # [7, F]
    scmap_row = np.repeat(s_pair_f, 4, axis=1)                    # [7, T]
    shared["scmap"] = np.ascontiguousarray(
        np.broadcast_to(scmap_row[None], (128, 7, T)).astype(np.float32))

    offsets = FIELD_DIM * np.arange(F, dtype=np.int64)
    in_maps = []
    for c in range(N_CORES):
        xs = np.asarray(x[c * BC:(c + 1) * BC]).astype(np.int64)  # [BC, F]
        im = dict(shared)
        if cw_on_device:
            # x16[p, f, j4] = x[16*j4 + p%16, f]
            xw = xs.astype(np.int16).reshape(BC // 16, 16, F)     # [j4, r, F]
            x16 = np.tile(xw.transpose(1, 2, 0), (8, 1, 1))       # [128, F, 32]
            im["x16"] = np.ascontiguousarray(x16)
        else:
            xg = (xs + offsets[None, :]).astype(np.int64)
            xg_n = np.ascontiguousarray(xg.T).reshape(N)          # n = f*BC+b
            cw = idxcat[xg_n].T                                   # [15, N] int16
            wrap = cw.reshape(NS, NJ, 16).transpose(2, 0, 1)      # [16, NS, NJ]
            im["iall"] = np.ascontiguousarray(np.tile(wrap, (8, 1, 1)))
        in_maps.append(im)
    return in_maps


def unshard(outs):
    """outs: list of per-core {'out': [128, T*D]} -> full (B, F, D) f32."""
    parts = []
    for c in range(N_CORES):
        o = outs[c]["out"].reshape(128, T, D).transpose(1, 0, 2)  # [T, 128, D]
        o = o.reshape(F, BC, D).transpose(1, 0, 2)                # [BC, F, D]
        parts.append(o)
    return np.ascontiguousarray(np.concatenate(parts, axis=0))


def kernel(**inputs):
    from concourse.bass_utils import run_bass_kernel_spmd
    nc = build_program()
    in_maps = host_prep(inputs)
    res = run_bass_kernel_spmd(nc, in_maps, core_ids=list(range(N_CORES)))
    return unshard(res.results)


# revision 17
# speedup vs baseline: 1.0186x; 1.0186x over previous
"""Trainium2 Bass kernel for nn_MixSizeNumQuatEmbedding (vq_codebook).

Data-parallel over batch across 8 NeuronCores. Per core (512 batch rows,
N = 512*39 = 19968 lookups, lookup order n = f*512 + b):

  stage A (codeword ids): one dma_gather per field pulls the 256B-padded
    row of the column-concatenated index table (int16) for each of the
    field's 512 global feature ids (x < 10000 fits int16); an SBUF->SBUF
    fold DMA plus per-column DVE copies rewrap the 15 codeword-id streams
    into dma_gather's 16-partition-interleaved index format. (Optional:
    cw_on_device=False moves this stage to host numpy.)

  stage B (embedding gather): per (codebook, subvector) stream s, chunked
    dma_gathers pull one subvector row slice per lookup straight from HBM
    (host uploads per-subvector codebook column slices so every gathered
    element is a legal 256B/512B row).

  stage C: DVE multiplies each gathered slice by its per-field arch_prob
    weight (free-dim broadcast AP) and accumulates the 7 codebook
    contributions into a [128, Tc, 128] tile, stored contiguously per
    partition to DRAM.
"""

import numpy as np

import concourse.bacc as bacc
import concourse.bass as bass
import concourse.mybir as mybir
import concourse.tile as tile

# Problem constants (hardcoded per harness contract).
B, F, D = 4096, 39, 128
FIELD_DIM = 10000
N_CORES = 8
BC = B // N_CORES            # 512 batch rows per core
N = BC * F                   # 19968 lookups per core
T = N // 128                 # 156 slots of 128 lookups
NJ = N // 16                 # 1248 wrapped-index columns
G = F * FIELD_DIM            # 390000 global feature ids

PAIRS = [(0, 0), (1, 0), (1, 1), (1, 2), (2, 0), (2, 1), (2, 2)]
M_SPACE = [1, 2, 4]
ROWS = {0: 19500, 1: 9984, 2: 19968}
PAIR_ROWS = [ROWS[j] for (j, m) in PAIRS]
PAIR_MN = [M_SPACE[m] for (j, m) in PAIRS]
PAIR_APCOL = [j * 3 + m for (j, m) in PAIRS]

# 15 (codebook, subvector) gather streams. Stream s gathers `w` floats per
# lookup from its host-sliced codebook view hcb_s [rows, w]; the first
# `plen` are valid and scale-accumulate into acc[:, :, doff:doff+plen].
SLICES = []
for p, (j, m) in enumerate(PAIRS):
    mn = PAIR_MN[p]
    plen = D // mn
    w = 128 if mn == 1 else 64
    for i in range(mn):
        SLICES.append(dict(pair=p, sub=i, w=w, plen=plen, doff=i * plen))
NS = len(SLICES)
assert NS == 15

_CACHE = {}


def build_program(loop=0, cw_on_device=True, tc=8, nqueues=4, scratch=16384,
                  skip_fma=False, skip_store=False, gbufs=2, fc=8, fc0=4):
    """loop>0: wrap the body in a HW For_i loop (timing amplification).
    fc: fields per stage-A chunk; fc0: size of the first (ramp) chunk."""
    key = (loop, cw_on_device, tc, nqueues, scratch, skip_fma, skip_store,
           gbufs, fc, fc0)
    if key in _CACHE:
        return _CACHE[key]
    f32 = mybir.dt.float32
    i16 = mybir.dt.int16
    chunks = [(t0, min(tc, T - t0)) for t0 in range(0, T, tc)]
    nc = bacc.Bacc("TRN2", target_bir_lowering=False, debug=False,
                   num_devices=N_CORES, num_swdge_queues=nqueues,
                   dynamic_dma_scratch_size=scratch)

    hcb_d = [nc.dram_tensor(f"hcb{s}", [PAIR_ROWS[sl["pair"]], sl["w"]], f32,
                            kind="ExternalInput")
             for s, sl in enumerate(SLICES)]
    sc_d = nc.dram_tensor("scmap", [128, 7, T], f32, kind="ExternalInput")
    out_d = nc.dram_tensor("out", [128, T * D], f32, kind="ExternalOutput")
    if cw_on_device:
        # x16[p, f, j4] = x[b = 16*j4 + p%16, f] wrapped for per-field gathers
        x16_d = nc.dram_tensor("x16", [128, F, BC // 16], i16,
                               kind="ExternalInput")
        idx16_d = nc.dram_tensor("idx16", [G, 128], i16, kind="ExternalInput")
    else:
        iall_d = nc.dram_tensor("iall", [128, NS, NJ], i16,
                                kind="ExternalInput")

    from contextlib import ExitStack
    with tile.TileContext(nc) as tc_ctx, ExitStack() as ctx:
        cpool = ctx.enter_context(tc_ctx.tile_pool(name="const", bufs=1))
        gpool = ctx.enter_context(tc_ctx.tile_pool(name="g", bufs=gbufs))
        apool = ctx.enter_context(tc_ctx.tile_pool(name="acc", bufs=2))
        tpool = ctx.enter_context(tc_ctx.tile_pool(name="tmp", bufs=2))

        sc_sb = cpool.tile([128, 7, T], f32)
        nc.sync.dma_start(out=sc_sb[:], in_=sc_d.ap())

        iall = cpool.tile([128, NS, NJ], i16)
        if cw_on_device:
            x16 = cpool.tile([128, F, BC // 16], i16)
            nc.sync.dma_start(out=x16[:], in_=x16_d.ap())
        else:
            nc.sync.dma_start(out=iall[:], in_=iall_d.ap())

        out_ap = out_d.ap().rearrange("p (t d) -> p t d", d=D)

        FC = fc                 # fields per stage-A chunk (SBUF economy)
        FCS = []
        _f = 0
        while _f < F:
            _n = fc0 if _f == 0 else FC
            FCS.append((_f, min(_f + _n, F)))
            _f += _n

        def stage_a_chunk(fc, fe):
            nf = fe - fc
            # cw: per field f, position b -> partition b%128, slot b//128
            cw = cpool.tile([128, FC, 4, 128], i16, tag="cw")
            for fi in range(nf):
                f = fc + fi
                nc.gpsimd.dma_gather(
                    out_ap=cw[:, fi, :, :],
                    in_ap=idx16_d.ap()[f * FIELD_DIM:(f + 1) * FIELD_DIM, :],
                    idxs_ap=x16[:, f, :],
                    num_idxs=BC, num_idxs_reg=BC, elem_size=128,
                    queue_num=f % nqueues)
            # fold 16g+r -> r: fd[r, fi, s4, g, c] = cw[16g+r, fi, s4, c]
            fd = cpool.tile([128, FC, 4, 8, 16], i16, tag="fd")
            for g in range(8):
                nc.sync.dma_start(
                    out=fd[0:16, 0:nf, :, g, :],
                    in_=cw[16 * g:16 * (g + 1), 0:nf, :, 0:16])
            # wrapped index rows: iall[r, s, 32f+8s4+g] = cw_s(16j+r)
            fdv = fd[0:16, 0:nf, :, :, :].rearrange("r f a g c -> r (f a g) c")
            for s in range(NS):
                nc.vector.tensor_copy(
                    out=iall[0:16, s, fc * 32:fe * 32], in_=fdv[:, :, s])
            # replicate this fields-slice to the other 7 16-partition groups
            # (tree doubling); slice-level so stage B of this range can start
            # while the next field-chunk's stage A still runs.
            for span in (16, 32, 64):
                nc.sync.dma_start(
                    out=iall[span:2 * span, :, fc * 32:fe * 32],
                    in_=iall[0:span, :, fc * 32:fe * 32])

        def stage_b_range(t_lo, t_hi):
            for t0 in range(t_lo, t_hi, tc):
              tc_ = min(tc, t_hi - t0)
              gts = []
              for s, sl in enumerate(SLICES):
                gt = gpool.tile([128, tc, sl["w"]], f32, tag=f"g{s}")
                nc.gpsimd.dma_gather(
                    out_ap=gt[:, 0:tc_, :],
                    in_ap=hcb_d[s].ap(),
                    idxs_ap=iall[:, s, t0 * 8:(t0 + tc_) * 8],
                    num_idxs=tc_ * 128, num_idxs_reg=tc_ * 128,
                    elem_size=sl["w"], queue_num=s % nqueues)
                gts.append(gt)

              acc = apool.tile([128, tc, D], f32)
              for s, sl in enumerate(SLICES if not skip_fma else []):
                plen = sl["plen"]
                gv = gts[s][:, 0:tc_, 0:plen]
                sc_bc = (sc_sb[:, sl["pair"], t0:t0 + tc_]
                         .unsqueeze(2).to_broadcast([128, tc_, plen]))
                dst = acc[:, 0:tc_, sl["doff"]:sl["doff"] + plen]
                if s == 0:
                    nc.vector.tensor_tensor(out=dst, in0=gv, in1=sc_bc,
                                            op=mybir.AluOpType.mult)
                else:
                    tmp = tpool.tile([128, tc, plen], f32, tag="tmp")
                    nc.vector.tensor_tensor(out=tmp[:, 0:tc_, :], in0=gv,
                                            in1=sc_bc,
                                            op=mybir.AluOpType.mult)
                    nc.vector.tensor_tensor(out=dst, in0=dst,
                                            in1=tmp[:, 0:tc_, :],
                                            op=mybir.AluOpType.add)
              if not skip_store and not skip_fma:
                nc.sync.dma_start(out=out_ap[:, t0:t0 + tc_, :],
                                  in_=acc[:, 0:tc_, :])
              elif not skip_store:
                # keep the output written so the program has live results
                nc.sync.dma_start(
                    out=out_ap[:, t0:t0 + tc_, :],
                    in_=gts[1][:, 0:tc_, :])

        def body():
            if not cw_on_device:
                stage_b_range(0, T)
                return
            # software pipeline: stage A of chunk i+1 overlaps stage B of i
            prev = None
            for (fc, fe) in FCS:
                stage_a_chunk(fc, fe)
                if prev is not None:
                    stage_b_range(4 * prev[0], 4 * prev[1])
                prev = (fc, fe)
            stage_b_range(4 * prev[0], min(4 * prev[1], T))

        if loop:
            with tc_ctx.For_i(0, loop, 1):
                body()
        else:
            body()

    nc.compile()
    _CACHE[key] = nc
    return nc


def host_prep(inputs, cw_on_device=True):
    """Build per-core in_maps from the full problem inputs."""
    x = np.asarray(inputs["x"])
    arch_prob = np.asarray(inputs["arch_prob"], dtype=np.float32)

    idx_cols = []
    for (j, m) in PAIRS:
        idx_cols.append(np.asarray(inputs[f"idx_{j}_{m}"]).astype(np.int16))
    idxcat = np.concatenate(idx_cols, axis=1)                     # [G, 15]

    shared = {}
    if cw_on_device:
        idx16 = np.zeros((G, 128), np.int16)
        idx16[:, :15] = idxcat
        shared["idx16"] = idx16

    for s, sl in enumerate(SLICES):
        (j, m) = PAIRS[sl["pair"]]
        cb = np.asarray(inputs[f"cb_{j}_{m}"]).astype(np.float32)
        mn = PAIR_MN[sl["pair"]]
        i, plen, w = sl["sub"], sl["plen"], sl["w"]
        if mn == 1:
            hv = cb
        else:
            hv = np.zeros((cb.shape[0], w), np.float32)
            take = min(w, D - i * plen)
            hv[:, :take] = cb[:, i * plen:i * plen + take]
        shared[f"hcb{s}"] = np.ascontiguousarray(hv)

    # scale map: scmap[p, pair, t] = arch_prob[t//4, apcol(pair)]
    s_pair_f = arch_prob[:, PAIR_APCOL].T.astype(np.float32)      # [7, F]
    scmap_row = np.repeat(s_pair_f, 4, axis=1)                    # [7, T]
    shared["scmap"] = np.ascontiguousarray(
        np.broadcast_to(scmap_row[None], (128, 7, T)).astype(np.float32))

    offsets = FIELD_DIM * np.arange(F, dtype=np.int64)
    in_maps = []
    for c in range(N_CORES):
        xs = np.asarray(x[c * BC:(c + 1) * BC]).astype(np.int64)  # [BC, F]
        im = dict(shared)
        if cw_on_device:
            # x16[p, f, j4] = x[16*j4 + p%16, f]
            xw = xs.astype(np.int16).reshape(BC // 16, 16, F)     # [j4, r, F]
            x16 = np.tile(xw.transpose(1, 2, 0), (8, 1, 1))       # [128, F, 32]
            im["x16"] = np.ascontiguousarray(x16)
        else:
            xg = (xs + offsets[None, :]).astype(np.int64)
            xg_n = np.ascontiguousarray(xg.T).reshape(N)          # n = f*BC+b
            cw = idxcat[xg_n].T                                   # [15, N] int16
            wrap = cw.reshape(NS, NJ, 16).transpose(2, 0, 1)      # [16, NS, NJ]
            im["iall"] = np.ascontiguousarray(np.tile(wrap, (8, 1, 1)))
        in_maps.append(im)
    return in_maps


def unshard(outs):
    """outs: list of per-core {'out': [128, T*D]} -> full (B, F, D) f32."""
    parts = []
    for c in range(N_CORES):
        o = outs[c]["out"].reshape(128, T, D).transpose(1, 0, 2)  # [T, 128, D]
        o = o.reshape(F, BC, D).transpose(1, 0, 2)                # [BC, F, D]
        parts.append(o)
    return np.ascontiguousarray(np.concatenate(parts, axis=0))


def kernel(**inputs):
    from concourse.bass_utils import run_bass_kernel_spmd
    nc = build_program()
    in_maps = host_prep(inputs)
    res = run_bass_kernel_spmd(nc, in_maps, core_ids=list(range(N_CORES)))
    return unshard(res.results)


# revision 24
# speedup vs baseline: 115.1334x; 113.0257x over previous
"""Trainium2 Bass kernel for nn_MixSizeNumQuatEmbedding (vq_codebook).

Data-parallel over batch across 8 NeuronCores. Per core (512 batch rows,
N = 512*39 = 19968 lookups, lookup order n = f*512 + b):

  stage A (codeword ids): one dma_gather per field pulls the 256B-padded
    row of the column-concatenated index table (int16) for each of the
    field's 512 global feature ids (x < 10000 fits int16); an SBUF->SBUF
    fold DMA plus per-column DVE copies rewrap the 15 codeword-id streams
    into dma_gather's 16-partition-interleaved index format. (Optional:
    cw_on_device=False moves this stage to host numpy.)

  stage B (embedding gather): per (codebook, subvector) stream s, chunked
    dma_gathers pull one subvector row slice per lookup straight from HBM
    (host uploads per-subvector codebook column slices so every gathered
    element is a legal 256B/512B row).

  stage C: DVE multiplies each gathered slice by its per-field arch_prob
    weight (free-dim broadcast AP) and accumulates the 7 codebook
    contributions into a [128, Tc, 128] tile, stored contiguously per
    partition to DRAM.
"""

import numpy as np

import concourse.bacc as bacc
import concourse.bass as bass
import concourse.mybir as mybir
import concourse.tile as tile

# Problem constants (hardcoded per harness contract).
B, F, D = 4096, 39, 128
FIELD_DIM = 10000
N_CORES = 8
BC = B // N_CORES            # 512 batch rows per core
N = BC * F                   # 19968 lookups per core
T = N // 128                 # 156 slots of 128 lookups
NJ = N // 16                 # 1248 wrapped-index columns
G = F * FIELD_DIM            # 390000 global feature ids

PAIRS = [(0, 0), (1, 0), (1, 1), (1, 2), (2, 0), (2, 1), (2, 2)]
M_SPACE = [1, 2, 4]
ROWS = {0: 19500, 1: 9984, 2: 19968}
PAIR_ROWS = [ROWS[j] for (j, m) in PAIRS]
PAIR_MN = [M_SPACE[m] for (j, m) in PAIRS]
PAIR_APCOL = [j * 3 + m for (j, m) in PAIRS]

# 15 (codebook, subvector) gather streams. Stream s gathers `w` floats per
# lookup from its host-sliced codebook view hcb_s [rows, w]; the first
# `plen` are valid and scale-accumulate into acc[:, :, doff:doff+plen].
def make_slices(w2=64):
    """w2: gather width (floats) for mn=2 streams (64 = 256B, 128 = 512B)."""
    sl = []
    for p, (j, m) in enumerate(PAIRS):
        mn = PAIR_MN[p]
        plen = D // mn
        w = 128 if mn == 1 else (w2 if mn == 2 else 64)
        for i in range(mn):
            sl.append(dict(pair=p, sub=i, w=w, plen=plen, doff=i * plen))
    return sl

SLICES = make_slices()
NS = len(SLICES)
assert NS == 15

_CACHE = {}


def build_program(loop=0, cw_on_device=True, tc=8, nqueues=4, scratch=16384,
                  skip_fma=False, skip_store=False, gbufs=2, fc=8, fc0=4,
                  w2=64, fold_eng="sync", abufs=2):
    """loop>0: wrap the body in a HW For_i loop (timing amplification).
    fc: fields per stage-A chunk; fc0: size of the first (ramp) chunk."""
    key = (loop, cw_on_device, tc, nqueues, scratch, skip_fma, skip_store,
           gbufs, fc, fc0, w2, fold_eng, abufs)
    if key in _CACHE:
        return _CACHE[key]
    f32 = mybir.dt.float32
    i16 = mybir.dt.int16
    chunks = [(t0, min(tc, T - t0)) for t0 in range(0, T, tc)]
    slices = make_slices(w2)
    nc = bacc.Bacc("TRN2", target_bir_lowering=False, debug=False,
                   num_devices=N_CORES, num_swdge_queues=nqueues,
                   dynamic_dma_scratch_size=scratch)

    hcb_d = [nc.dram_tensor(f"hcb{s}", [PAIR_ROWS[sl["pair"]], sl["w"]], f32,
                            kind="ExternalInput")
             for s, sl in enumerate(slices)]
    sc_d = nc.dram_tensor("scmap", [128, 7, T], f32, kind="ExternalInput")
    out_d = nc.dram_tensor("out", [128, T * D], f32, kind="ExternalOutput")
    if cw_on_device:
        # x16[p, f, j4] = x[b = 16*j4 + p%16, f] wrapped for per-field gathers
        x16_d = nc.dram_tensor("x16", [128, F, BC // 16], i16,
                               kind="ExternalInput")
        idx16_d = nc.dram_tensor("idx16", [G, 128], i16, kind="ExternalInput")
    else:
        iall_d = nc.dram_tensor("iall", [128, NS, NJ], i16,
                                kind="ExternalInput")

    from contextlib import ExitStack
    fold_dma = (nc.scalar.dma_start if fold_eng == "scalar"
                else nc.sync.dma_start)
    with tile.TileContext(nc) as tc_ctx, ExitStack() as ctx:
        cpool = ctx.enter_context(tc_ctx.tile_pool(name="const", bufs=1))
        gpool = ctx.enter_context(tc_ctx.tile_pool(name="g", bufs=gbufs))
        # wide (512B-desc) streams single-buffered to fit SBUF at tc=15
        g1pool = (ctx.enter_context(tc_ctx.tile_pool(name="g1", bufs=1))
                  if tc > 8 and gbufs > 1 else gpool)
        apool = ctx.enter_context(tc_ctx.tile_pool(name="acc", bufs=abufs))
        tpool = ctx.enter_context(tc_ctx.tile_pool(name="tmp", bufs=abufs))

        sc_sb = cpool.tile([128, 7, T], f32)
        nc.sync.dma_start(out=sc_sb[:], in_=sc_d.ap())

        iall = cpool.tile([128, NS, NJ], i16)
        if cw_on_device:
            x16 = cpool.tile([128, F, BC // 16], i16)
            nc.sync.dma_start(out=x16[:], in_=x16_d.ap())
        else:
            nc.sync.dma_start(out=iall[:], in_=iall_d.ap())

        out_ap = out_d.ap().rearrange("p (t d) -> p t d", d=D)

        FC = fc                 # fields per stage-A chunk (SBUF economy)
        FCS = []
        _f = 0
        while _f < F:
            _n = fc0 if _f == 0 else FC
            FCS.append((_f, min(_f + _n, F)))
            _f += _n

        def stage_a_chunk(fc, fe):
            nf = fe - fc
            # cw: per field f, position b -> partition b%128, slot b//128
            cw = cpool.tile([128, FC, 4, 128], i16, tag="cw")
            for fi in range(nf):
                f = fc + fi
                nc.gpsimd.dma_gather(
                    out_ap=cw[:, fi, :, :],
                    in_ap=idx16_d.ap()[f * FIELD_DIM:(f + 1) * FIELD_DIM, :],
                    idxs_ap=x16[:, f, :],
                    num_idxs=BC, num_idxs_reg=BC, elem_size=128,
                    queue_num=f % nqueues)
            # fold 16g+r -> r: fd[r, fi, s4, g, c] = cw[16g+r, fi, s4, c]
            fd = cpool.tile([128, FC, 4, 8, 16], i16, tag="fd")
            for g in range(8):
                fold_dma(
                    out=fd[0:16, 0:nf, :, g, :],
                    in_=cw[16 * g:16 * (g + 1), 0:nf, :, 0:16])
            # wrapped index rows: iall[r, s, 32f+8s4+g] = cw_s(16j+r)
            fdv = fd[0:16, 0:nf, :, :, :].rearrange("r f a g c -> r (f a g) c")
            for s in range(NS):
                nc.vector.tensor_copy(
                    out=iall[0:16, s, fc * 32:fe * 32], in_=fdv[:, :, s])
            # replicate this fields-slice to the other 7 16-partition groups
            # (tree doubling); slice-level so stage B of this range can start
            # while the next field-chunk's stage A still runs.
            for span in (16, 32, 64):
                fold_dma(
                    out=iall[span:2 * span, :, fc * 32:fe * 32],
                    in_=iall[0:span, :, fc * 32:fe * 32])

        def stage_b_range(t_lo, t_hi):
            for t0 in range(t_lo, t_hi, tc):
              tc_ = min(tc, t_hi - t0)
              emit_chunk(t0, tc_)

        def emit_chunk(t0, tc_):
              gts = []
              for s, sl in enumerate(slices):
                pool_s = g1pool if sl["w"] == 128 else gpool
                gt = pool_s.tile([128, tc, sl["w"]], f32, tag=f"g{s}")
                nc.gpsimd.dma_gather(
                    out_ap=gt[:, 0:tc_, :],
                    in_ap=hcb_d[s].ap(),
                    idxs_ap=iall[:, s, t0 * 8:(t0 + tc_) * 8],
                    num_idxs=tc_ * 128, num_idxs_reg=tc_ * 128,
                    elem_size=sl["w"], queue_num=s % nqueues)
                gts.append(gt)

              acc = apool.tile([128, tc, D], f32)
              for s, sl in enumerate(slices if not skip_fma else []):
                plen = sl["plen"]
                gv = gts[s][:, 0:tc_, 0:plen]
                sc_bc = (sc_sb[:, sl["pair"], t0:t0 + tc_]
                         .unsqueeze(2).to_broadcast([128, tc_, plen]))
                dst = acc[:, 0:tc_, sl["doff"]:sl["doff"] + plen]
                if s == 0:
                    nc.vector.tensor_tensor(out=dst, in0=gv, in1=sc_bc,
                                            op=mybir.AluOpType.mult)
                else:
                    tmp = tpool.tile([128, tc, plen], f32, tag="tmp")
                    nc.vector.tensor_tensor(out=tmp[:, 0:tc_, :], in0=gv,
                                            in1=sc_bc,
                                            op=mybir.AluOpType.mult)
                    nc.vector.tensor_tensor(out=dst, in0=dst,
                                            in1=tmp[:, 0:tc_, :],
                                            op=mybir.AluOpType.add)
              if not skip_store and not skip_fma:
                nc.sync.dma_start(out=out_ap[:, t0:t0 + tc_, :],
                                  in_=acc[:, 0:tc_, :])
              elif not skip_store:
                # keep the output written so the program has live results
                nc.sync.dma_start(
                    out=out_ap[:, t0:t0 + tc_, :],
                    in_=gts[1][:, 0:tc_, :])

        def body():
            if not cw_on_device:
                stage_b_range(0, T)
                return
            # software pipeline: stage A of chunk i+1 overlaps stage B of i.
            # Stage-B chunks run on a global tc-slot grid decoupled from the
            # stage-A field-chunk boundaries (Tile's subtile deps let a chunk
            # read iall slices written by two different A-chunks).
            emitted = 0     # next stage-B slot to emit
            avail = 0       # slots covered by A-chunks completed BEFORE the
                            # one currently being emitted (one-chunk lag)
            for (fc, fe) in FCS:
                stage_a_chunk(fc, fe)
                while emitted + tc <= avail:
                    emit_chunk(emitted, tc)
                    emitted += tc
                avail = 4 * fe
            while emitted < T:
                w = min(tc, T - emitted)
                emit_chunk(emitted, w)
                emitted += w

        if loop:
            with tc_ctx.For_i(0, loop, 1):
                body()
        else:
            body()

    nc.compile()
    _CACHE[key] = nc
    return nc


def host_prep(inputs, cw_on_device=True, w2=64):
    """Build per-core in_maps from the full problem inputs."""
    x = np.asarray(inputs["x"])
    arch_prob = np.asarray(inputs["arch_prob"], dtype=np.float32)

    idx_cols = []
    for (j, m) in PAIRS:
        idx_cols.append(np.asarray(inputs[f"idx_{j}_{m}"]).astype(np.int16))
    idxcat = np.concatenate(idx_cols, axis=1)                     # [G, 15]

    shared = {}
    if cw_on_device:
        idx16 = np.zeros((G, 128), np.int16)
        idx16[:, :15] = idxcat
        shared["idx16"] = idx16

    for s, sl in enumerate(make_slices(w2)):
        (j, m) = PAIRS[sl["pair"]]
        cb = np.asarray(inputs[f"cb_{j}_{m}"]).astype(np.float32)
        mn = PAIR_MN[sl["pair"]]
        i, plen, w = sl["sub"], sl["plen"], sl["w"]
        if mn == 1:
            hv = cb
        else:
            hv = np.zeros((cb.shape[0], w), np.float32)
            take = min(w, D - i * plen)
            hv[:, :take] = cb[:, i * plen:i * plen + take]
        shared[f"hcb{s}"] = np.ascontiguousarray(hv)

    # scale map: scmap[p, pair, t] = arch_prob[t//4, apcol(pair)]
    s_pair_f = arch_prob[:, PAIR_APCOL].T.astype(np.float32)      # [7, F]
    scmap_row = np.repeat(s_pair_f, 4, axis=1)                    # [7, T]
    shared["scmap"] = np.ascontiguousarray(
        np.broadcast_to(scmap_row[None], (128, 7, T)).astype(np.float32))

    offsets = FIELD_DIM * np.arange(F, dtype=np.int64)
    in_maps = []
    for c in range(N_CORES):
        xs = np.asarray(x[c * BC:(c + 1) * BC]).astype(np.int64)  # [BC, F]
        im = dict(shared)
        if cw_on_device:
            # x16[p, f, j4] = x[16*j4 + p%16, f]
            xw = xs.astype(np.int16).reshape(BC // 16, 16, F)     # [j4, r, F]
            x16 = np.tile(xw.transpose(1, 2, 0), (8, 1, 1))       # [128, F, 32]
            im["x16"] = np.ascontiguousarray(x16)
        else:
            xg = (xs + offsets[None, :]).astype(np.int64)
            xg_n = np.ascontiguousarray(xg.T).reshape(N)          # n = f*BC+b
            cw = idxcat[xg_n].T                                   # [15, N] int16
            wrap = cw.reshape(NS, NJ, 16).transpose(2, 0, 1)      # [16, NS, NJ]
            im["iall"] = np.ascontiguousarray(np.tile(wrap, (8, 1, 1)))
        in_maps.append(im)
    return in_maps


def unshard(outs):
    """outs: list of per-core {'out': [128, T*D]} -> full (B, F, D) f32."""
    parts = []
    for c in range(N_CORES):
        o = outs[c]["out"].reshape(128, T, D).transpose(1, 0, 2)  # [T, 128, D]
        o = o.reshape(F, BC, D).transpose(1, 0, 2)                # [BC, F, D]
        parts.append(o)
    return np.ascontiguousarray(np.concatenate(parts, axis=0))


def kernel(**inputs):
    from concourse.bass_utils import run_bass_kernel_spmd
    nc = build_program()
    in_maps = host_prep(inputs)
    res = run_bass_kernel_spmd(nc, in_maps, core_ids=list(range(N_CORES)))
    return unshard(res.results)
